# revision 1
# baseline (speedup 1.0000x reference)
"""Trainium2 Bass kernel for nn_MultiHeadAttention_37512244363503.

Sharding: 8 cores = 4 batches x 2 head-groups (8 heads each).
Per core (b, g): Wq/Wk/Wv column-sliced, Wo row-sliced; the host sums the
two partial outputs per batch (the row-parallel "all-reduce") and adds bo.

Key compaction: pad_mask is host-visible, so masked keys are dropped on
the host before upload. Keys compact to jt*128 columns (jt chosen from
the max per-batch unmasked count, 5 for the reference distribution),
cutting K/V projections, scores, softmax and AV by L_c/L. Padding slots
get a -30000 exp-bias so they contribute exactly 0.

All DRAM inputs are pre-tiled on host to the SBUF k-tile-major layout so
every load is a contiguous [128, cols] copy (no strided descriptors).

Per-core algorithm (matmuls bf16 in / fp32 PSUM accumulate):
  QT[d,i]   = Wq_g.T @ x_q[b].T        (d=512 cols of this group)
  KT[d,j]   = Wk_g.T @ x_kv_c[b].T     (j over compacted keys)
  V[j,c]    = x_kv_c[b] @ Wv_g         (per-head [V_h | ones*64] blocks)
  ST[j,i]   = K_h Q_h.T per head       (2 heads packed via PE row groups)
  PT        = exp(ST/8 + mask_bias[j]) (ACT; bias rides the ACT bias input)
  po        = V_ext.T @ PT             (rows 0:64 = OT, 64:128 = denom)
  O_norm^T  = po[0:64] * recip(po[64:128])   (ACT reciprocal + DVE mul)
  partial   = O_norm @ Wo_g            (bf16 partial -> DRAM, host sums)
"""

import numpy as np
import ml_dtypes

import concourse.bass as bass
import concourse.mybir as mybir
from concourse import bacc
from concourse.tile import TileContext
from concourse.bass_utils import run_bass_kernel_spmd

BF16 = ml_dtypes.bfloat16

B, N, L, D, H = 4, 1024, 1024, 1024, 16
DH = D // H           # 64 channels per head
HG = 8                # heads per core
DG = HG * DH          # 512 channels per core
NCORES = 8
DP_SCALE = DH ** -0.5
MASK_NEG = -30000.0   # exp(x + MASK_NEG) underflows to exactly 0.0

f32 = mybir.dt.float32
bf16 = mybir.dt.bfloat16

RECIP_FAST = True     # reciprocal_approx_fast vs plain reciprocal

KT = D // 128         # 8 k-tiles in the contraction dim of projections
IT = N // 128         # 8 query tiles
IC = N // 512         # 2 query chunks (PSUM free dim)
PAIRS = HG // 2       # 4 head pairs (2 heads packed per 128 partitions)
VW = 2 * DH           # 128 cols per (j, head) V_ext block: [V_h | ones]


def build_nc(jt=5, debug=False, num_devices=NCORES, repeat=1):
    lc = jt * 128
    nc = bacc.Bacc("TRN2", target_bir_lowering=False, debug=False,
                   num_devices=num_devices)

    xq = nc.dram_tensor("xq", [128, KT * N], bf16, kind="ExternalInput")
    xkv = nc.dram_tensor("xkv", [128, KT * lc], bf16, kind="ExternalInput")
    # wq/wk are PAIR-major ([128, pair*KT*128 + k*128 + c]) so one pair's
    # projection weights are a single contiguous 256KB slice loadable first
    wq = nc.dram_tensor("wq", [128, KT * DG], bf16, kind="ExternalInput")
    wk = nc.dram_tensor("wk", [128, KT * DG], bf16, kind="ExternalInput")
    wv = nc.dram_tensor("wv", [128, KT * DG], bf16, kind="ExternalInput")
    wo = nc.dram_tensor("wo", [128, PAIRS * D], bf16, kind="ExternalInput")
    mb = nc.dram_tensor("mb", [128, jt], f32, kind="ExternalInput")
    out = nc.dram_tensor("out", [N, D], bf16, kind="ExternalOutput")

    with TileContext(nc) as tc:
        with (
            tc.tile_pool(name="persist", bufs=1) as persist,
            tc.tile_pool(name="pt", bufs=2) as pt_pool,
            tc.tile_pool(name="recp", bufs=4) as recp,
            tc.tile_pool(name="scp", bufs=4) as scp,
            tc.tile_pool(name="stps", bufs=2, space="PSUM") as stps,
            tc.tile_pool(name="smps", bufs=2, space="PSUM") as smps,
        ):
            env = dict(jt=jt, lc=lc)
            for nm, shape in [
                ("xq_sb", [128, KT * N]), ("xkv_sb", [128, KT * lc]),
                ("wq_sb", [128, KT * DG]), ("wk_sb", [128, KT * DG]),
                ("wv_sb", [128, KT * DG]), ("wo_sb", [128, PAIRS * D]),
                ("qT_sb", [128, PAIRS * N]), ("kT_sb", [128, PAIRS * lc]),
                ("v_sb", [128, jt * HG * VW]), ("ot_sb", [128, PAIRS * N]),
            ]:
                env[nm] = persist.tile(shape, bf16, name=nm)
            env["mb_sb"] = persist.tile([128, jt], f32, name="mb_sb")
            env["v_view"] = env["v_sb"][:].rearrange(
                "p (j h c) -> p j h c", j=jt, h=HG)
            # ones half FIRST in each V_ext block: [ones | V_h]. This puts
            # the softmax denominator at PSUM partitions 0:64 so the custom
            # reciprocal reads it base-aligned (it corrupts on shifted APs),
            # and O^T at 64:128 rides the mul as the shiftable PSUM operand.
            nc.vector.memset(env["v_view"][:, :, :, 0:DH], 1.0)
            env.update(pt_pool=pt_pool, recp=recp, scp=scp, stps=stps,
                       smps=smps, xq=xq, xkv=xkv, wq=wq, wk=wk, wv=wv,
                       wo=wo, mb=mb, out=out)
            for _rep in range(repeat):
                _emit_body(nc, env)

    nc.compile()
    return nc


def _emit_body(nc, env):
    jt, lc = env["jt"], env["lc"]
    xq_sb, xkv_sb = env["xq_sb"], env["xkv_sb"]
    wq_sb, wk_sb, wv_sb, wo_sb = (env[k] for k in
                                  ["wq_sb", "wk_sb", "wv_sb", "wo_sb"])
    qT_sb, kT_sb, v_sb, ot_sb, mb_sb = (env[k] for k in
                                        ["qT_sb", "kT_sb", "v_sb", "ot_sb",
                                         "mb_sb"])
    v_view = env["v_view"]
    pt_pool, recp, scp = env["pt_pool"], env["recp"], env["scp"]
    stps, smps = env["stps"], env["smps"]
    xq, xkv, wq, wk, wv, wo, mb, out = (env[k] for k in
                                        ["xq", "xkv", "wq", "wk", "wv",
                                         "wo", "mb", "out"])

    # ---- input loads ----
    # Each dma_start lands on ONE hw queue at ~17 GB/s, so wide tensors
    # must be partition-split across several dma_starts to load in
    # parallel. Emission order = priority; the three DMA-capable engines
    # (sync/scalar/gpsimd) issue round-robin so early chunks enqueue fast.
    engs = [nc.sync, nc.scalar, nc.gpsimd]
    eidx = [0]

    def ld_split(dst, src, c0, c1, nsplit):
        step = 128 // nsplit
        for s in range(nsplit):
            p0, p1 = s * step, (s + 1) * step
            eng = engs[eidx[0] % 3]
            eidx[0] += 1
            eng.dma_start(out=dst[p0:p1, c0:c1], in_=src[p0:p1, c0:c1])

    nc.gpsimd.dma_start(out=mb_sb[:], in_=mb[:, :])
    PW = KT * 128  # cols per pair in the pair-major wq/wk layouts
    # priority order: everything pair-0's ST needs first (wq_p0, xq,
    # wk_p0, xkv), then later pairs' weights in first-use order
    ld_split(wq_sb, wq, 0, PW, 2)
    for k in range(KT):
        ld_split(xq_sb, xq, k * N, (k + 1) * N, 2)
    ld_split(wk_sb, wk, 0, PW, 2)
    ld_split(wq_sb, wq, PW, 2 * PW, 2)  # q1 runs in the prologue shadow
    hx = (KT // 2) * lc
    ld_split(xkv_sb, xkv, 0, hx, 2)
    ld_split(xkv_sb, xkv, hx, KT * lc, 2)
    ld_split(wv_sb, wv, 0, KT * DG, 4)
    ld_split(wk_sb, wk, PW, 2 * PW, 2)
    ld_split(wq_sb, wq, 2 * PW, 4 * PW, 2)
    ld_split(wk_sb, wk, 2 * PW, 4 * PW, 2)
    ld_split(wo_sb, wo, 0, PAIRS * D, 2)

    # k-proj output chunking (lc may exceed one PSUM bank)
    kch = [(0, min(512, lc))] + ([(512, lc)] if lc > 512 else [])

    def make_proj(dst_sb, w_sb, x_sb, p, cols, chunks, on_act=False):
        """Projection for pair p as two units sharing PSUM accumulators.

        chunks: list of (c0, c1) output-column ranges (<=512 wide each).
        Weights for k-tile are loaded once per k, streamed for each chunk.
        on_act: do the PSUM->SBUF copies on the idle ACT engine (prologue
        projections), keeping the DVE stream free.
        """
        ps = [None] * len(chunks)

        def half(k0, k1):
            for ci, (c0, c1) in enumerate(chunks):
                if k0 == 0:
                    ps[ci] = smps.tile([128, c1 - c0], f32,
                                       name="pj%d" % ci, tag="pj")
            for k in range(k0, k1):
                w = w_sb[:, (p * KT + k) * 128: (p * KT + k + 1) * 128]
                for ci, (c0, c1) in enumerate(chunks):
                    nc.tensor.matmul(
                        ps[ci][:],
                        lhsT=w,
                        rhs=x_sb[:, k * cols + c0: k * cols + c1],
                        start=(k == 0), stop=(k == KT - 1))
            if k1 == KT:
                for ci, (c0, c1) in enumerate(chunks):
                    dst = dst_sb[:, p * cols + c0: p * cols + c1]
                    if on_act:
                        nc.scalar.activation(
                            out=dst, in_=ps[ci][:],
                            func=mybir.ActivationFunctionType.Copy)
                    else:
                        nc.vector.tensor_copy(out=dst, in_=ps[ci][:])

        return [lambda: half(0, KT // 2), lambda: half(KT // 2, KT)]

    def proj_q(p, on_act=False):
        return make_proj(qT_sb, wq_sb, xq_sb, p, N, [(0, 512), (512, 1024)],
                         on_act)

    def proj_k(p, on_act=False):
        return make_proj(kT_sb, wk_sb, xkv_sb, p, lc, kch, on_act)

    def v_proj(j):
        """V[j, c] = x_kv @ Wv_g for one j tile."""
        ps = smps.tile([128, 512], f32, tag="av", bufs=1)
        for k in range(KT):
            nc.tensor.matmul(
                ps[:],
                lhsT=xkv_sb[:, k * lc + j * 128: k * lc + (j + 1) * 128],
                rhs=wv_sb[:, k * DG:(k + 1) * DG],
                start=(k == 0), stop=(k == KT - 1))
        nc.vector.tensor_copy(
            out=v_view[:, j, :, DH:VW],
            in_=ps[:].rearrange("p (h c) -> p h c", h=HG))

    def norm(po, p, hh):
        # po rows 0:64 = denom replicas, 64:128 = O^T (V_ext = [ones | V]).
        # denom > ~1 always (sum of exps over >=1 real key), so the fast
        # approx reciprocal's denorm/inf edge cases can't occur.
        rec_t = recp.tile([64, 1024], f32)
        if RECIP_FAST:
            nc.vector.reciprocal_approx_fast(out=rec_t[:], in_=po[0:64, :])
        else:
            nc.vector.reciprocal(out=rec_t[:], in_=po[0:64, :])
        sc_t = scp.tile([64, 1024], bf16, tag="sc")
        nc.vector.tensor_mul(out=sc_t[:], in0=po[64:128, :], in1=rec_t[:])
        # split across queues: the last pair's ot feeds the O projection,
        # so its store latency is on the critical path
        nsp = 4 if p == PAIRS - 1 else 2
        step = 64 // nsp
        for s in range(nsp):
            eng = (nc.sync, nc.gpsimd)[s % 2]
            eng.dma_start(
                out=ot_sb[hh * 64 + s * step: hh * 64 + (s + 1) * step,
                          p * N:(p + 1) * N],
                in_=sc_t[s * step:(s + 1) * step, :])

    def av_head(p, hh, pt, pool_tag="av"):
        """AV for head 2p+hh, both i chunks, one V weight load per j."""
        h = 2 * p + hh
        po = (stps.tile([128, 1024], f32, name="po", tag="st")
              if pool_tag == "st" else
              smps.tile([128, 1024], f32, name="po", tag="av", bufs=1))
        for j in range(jt):
            vblk = v_sb[:, (j * HG + h) * VW: (j * HG + h + 1) * VW]
            nc.tensor.matmul(po[:, 0:512], lhsT=vblk,
                             rhs=pt[:, j * N: j * N + 512],
                             start=(j == 0), stop=(j == jt - 1))
            nc.tensor.matmul(po[:, 512:1024], lhsT=vblk,
                             rhs=pt[:, j * N + 512: (j + 1) * N],
                             start=(j == 0), stop=(j == jt - 1))
        norm(po, p, hh)

    def st_pair(p, pa, pb, slot_units):
        """ST + exp for pair p; filler units interleaved into j slots."""
        for j in range(jt):
            for rb, pt in ((0, pa), (64, pb)):
                ps = stps.tile([128, 1024], f32, name="st", tag="st")
                kk = kT_sb[rb:rb + 64, p * lc + j * 128: p * lc + (j + 1) * 128]
                for ic in range(IC):
                    cols = slice(ic * 512, ic * 512 + 512)
                    nc.tensor.matmul(
                        ps[:, cols], lhsT=kk,
                        rhs=qT_sb[rb:rb + 64,
                                  p * N + ic * 512: p * N + ic * 512 + 512],
                        start=True, stop=True)
                # EXP right after this half's matmuls: ACT starts earlier
                # and the pool slot frees a half-j sooner
                nc.scalar.activation(
                    out=pt[:, j * N:(j + 1) * N], in_=ps[:],
                    func=mybir.ActivationFunctionType.Exp,
                    bias=mb_sb[:, j:j + 1], scale=DP_SCALE)
            for u in slot_units.get(j, []):
                u()

    def op_mms(it, ps0, ps1, cts):
        for ct in cts:
            ot_blk = ot_sb[:, ct * N + it * 128: ct * N + (it + 1) * 128]
            nc.tensor.matmul(
                ps0[:], lhsT=ot_blk,
                rhs=wo_sb[:, ct * D: ct * D + 512],
                start=(ct == 0), stop=(ct == PAIRS - 1))
            nc.tensor.matmul(
                ps1[:], lhsT=ot_blk,
                rhs=wo_sb[:, ct * D + 512: ct * D + 1024],
                start=(ct == 0), stop=(ct == PAIRS - 1))

    def op_finish(it, ps0, ps1):
        # copies split across DVE+ACT (both idle here); stores split
        # 4-way so the last tile's 256KB isn't one ~7.5us queue transfer
        out_t = scp.tile([128, 1024], bf16, tag="outt")
        nc.vector.tensor_copy(out=out_t[:, 0:512], in_=ps0[:])
        nc.scalar.activation(out=out_t[:, 512:1024], in_=ps1[:],
                             func=mybir.ActivationFunctionType.Copy)
        for s in range(4):
            eng = (nc.sync, nc.gpsimd)[s % 2]
            eng.dma_start(
                out=out[it * 128 + s * 32: it * 128 + (s + 1) * 32, :],
                in_=out_t[s * 32:(s + 1) * 32, :])

    op_state = {}

    def op_head_unit():
        # i-tile 0, pairs 0-1: runs as a pair-3 filler in the freed
        # projection PSUM banks
        ps0 = smps.tile([128, 512], f32, name="o0", tag="pj")
        ps1 = smps.tile([128, 512], f32, name="o1", tag="pj")
        op_state[0] = (ps0, ps1)
        op_mms(0, ps0, ps1, range(2))

    # ---- prologue projections: q0, q1, k0 (DMA-paced; q1 fills the PE
    # while xkv/wk stream in) ----
    for u in proj_q(0, on_act=True):
        u()
    for u in proj_q(1, on_act=True):
        u()
    for u in proj_k(0, on_act=True):
        u()

    # ---- pipelined pairs ----
    prev = None
    for p in range(PAIRS):
        pa = pt_pool.tile([128, jt * N], bf16, tag="pa")
        pb = pt_pool.tile([128, jt * N], bf16, tag="pb")

        proj_units = []
        if p + 1 < PAIRS:
            proj_units = list(proj_k(p + 1))
            if p + 2 < PAIRS:
                proj_units = proj_q(p + 2) + proj_units
        if p == PAIRS - 1:
            proj_units = [op_head_unit]
        if prev is not None:
            # av heads use the single "av" PSUM buffer: keep them apart so
            # head B's alloc never stalls the PE on head A's norms
            pp, ppa, ppb = prev
            na = len(proj_units) // 2
            units = ([lambda: av_head(pp, 0, ppa)] + proj_units[:na]
                     + [lambda: av_head(pp, 1, ppb)] + proj_units[na:])
        else:
            units = proj_units
        if p == 0:
            units += [lambda j=j: v_proj(j) for j in range(jt)]

        slot_units = {}
        for i, u in enumerate(units):
            # monotonic slot assignment keeps each proj's k0-half before
            # its k1-half (they share PSUM accumulators)
            slot_units.setdefault(i * jt // len(units), []).append(u)
        st_pair(p, pa, pb, slot_units)
        prev = (p, pa, pb)

    # last pair's AV + norm (head B from the free ST banks so it doesn't
    # wait on head A's norm to release the single av buffer)
    pp, ppa, ppb = prev
    av_head(pp, 0, ppa)
    av_head(pp, 1, ppb, pool_tag="st")

    # ---- output projection: partial[i, d] in bf16 ----
    # ot block is the stationary operand, shared across both d-chunks

    # i-tile 0 already holds pairs 0-1 (pair-3 filler); add pair 2 now,
    # then stagger i-tiles 1-2 (pairs 0-2) as runway before any
    # pair-3-dependent matmul, hiding the last norm + ot-store latency
    ps0, ps1 = op_state[0]
    op_mms(0, ps0, ps1, range(2, 3))
    pend = {}
    for it in (1, 2):
        pw = stps.tile([128, 1024], f32, name="pw", tag="st")
        pend[it] = (pw[:, 0:512], pw[:, 512:1024])
        op_mms(it, *pend[it], range(PAIRS - 1))
    op_mms(0, ps0, ps1, range(PAIRS - 1, PAIRS))
    op_finish(0, ps0, ps1)
    for it in range(1, IT):
        if it in pend:
            ps0, ps1 = pend[it]
            op_mms(it, ps0, ps1, range(PAIRS - 1, PAIRS))
        else:
            pw = stps.tile([128, 1024], f32, name="pw", tag="st")
            ps0 = pw[:, 0:512]
            ps1 = pw[:, 512:1024]
            op_mms(it, ps0, ps1, range(PAIRS))
        op_finish(it, ps0, ps1)


_NC_CACHE = {}


def _get_nc(jt):
    if jt not in _NC_CACHE:
        _NC_CACHE[jt] = build_nc(jt=jt)
    return _NC_CACHE[jt]


def _tile_k(a, cols):
    """[KT*128, cols] -> [128, KT*cols] k-tile-major, contiguous bf16."""
    return np.ascontiguousarray(
        a.reshape(KT, 128, cols).transpose(1, 0, 2).reshape(128, KT * cols)
    ).astype(BF16)


def _make_in_maps(x_q, x_kv, pad_mask, Wq, Wk, Wv, Wo, jt=None):
    pad_mask = np.asarray(pad_mask)
    cnts = (~pad_mask).sum(axis=1)
    if jt is None:
        jt = max(1, int(-(-int(cnts.max()) // 128)))
    lc = jt * 128

    def _tile_pair(w):
        # [D, DG] -> [128, pair*KT*128 + k*128 + c] pair-major
        return np.ascontiguousarray(
            w.reshape(KT, 128, PAIRS, 128).transpose(1, 2, 0, 3)
            .reshape(128, PAIRS * KT * 128)).astype(BF16)

    per_g = []
    for g in range(2):
        cols = slice(g * DG, (g + 1) * DG)
        per_g.append({
            "wq": _tile_pair(np.ascontiguousarray(Wq[:, cols])),
            "wk": _tile_pair(np.ascontiguousarray(Wk[:, cols])),
            "wv": _tile_k(np.ascontiguousarray(Wv[:, cols]), DG),
            "wo": np.ascontiguousarray(
                Wo[g * DG:(g + 1) * DG, :]
                .reshape(PAIRS, 128, D).transpose(1, 0, 2)
                .reshape(128, PAIRS * D)).astype(BF16),
        })
    per_b = []
    for b in range(B):
        idx = np.flatnonzero(~pad_mask[b])
        n = len(idx)
        xc = np.zeros((lc, D), dtype=np.float32)
        xc[:n] = x_kv[b][idx]
        mbias = np.full(lc, MASK_NEG, dtype=np.float32)
        mbias[:n] = 0.0
        per_b.append({
            "xq": _tile_k(np.ascontiguousarray(x_q[b].T), N),
            "xkv": _tile_k(np.ascontiguousarray(xc.T), lc),
            "mb": np.ascontiguousarray(mbias.reshape(jt, 128).T),
        })

    in_maps = []
    for c in range(NCORES):
        b, g = c // 2, c % 2
        in_maps.append({**per_b[b], **per_g[g]})
    return in_maps, jt


def kernel(x_q, x_kv, pad_mask, Wq, Wk, Wv, Wo, bo):
    in_maps, jt = _make_in_maps(x_q, x_kv, pad_mask, Wq, Wk, Wv, Wo)
    nc = _get_nc(jt)
    res = run_bass_kernel_spmd(nc, in_maps, core_ids=list(range(NCORES)))
    full = np.empty((B, N, D), dtype=np.float32)
    bo32 = bo.astype(np.float32)
    for b in range(B):
        full[b] = (res.results[2 * b]["out"].astype(np.float32)
                   + res.results[2 * b + 1]["out"].astype(np.float32))
        full[b] += bo32
    return full

